# revision 1
# baseline (speedup 1.0000x reference)
"""Trainium2 Bass kernel for nn_ACFModule (pooled-KV mixture attention block).

Sharding: 8 cores; core c owns batch b=c//4 and query rows 14*(c%4)..14*(c%4)+14
(784 of 3136 positions). KV (pooled) is computed per-core (replicated within a
batch group). Output is the core's [512, 784] slice; host reassembles.

All heavy math on device in bf16 (PSUM f32); softmax exp on ScalarE with
accumulated row-sums; mixture combine via beta=pi/Z on VectorE; E transposed
via DMA-xbar; AV + output projection on TensorE; BN+residual fused into the
projection evacuation.
"""

import sys
import time
from contextlib import ExitStack

import numpy as np

sys.path.insert(0, "/opt/trn_rl_repo")

import ml_dtypes  # noqa: E402

import concourse.bass as bass  # noqa: E402
from concourse import bacc  # noqa: E402
import concourse.mybir as mybir  # noqa: E402
import concourse.tile as tile  # noqa: E402

BF16 = ml_dtypes.bfloat16

# problem constants
B, C, Himg, Wimg = 2, 512, 56, 56
N = Himg * Wimg  # 3136
NH, NM, DK, DV = 8, 2, 64, 64
d = DK // NM  # 32
H2 = W2 = 28
N2 = H2 * W2  # 784
BN_EPS = 1e-5
TEMP = float(DK) ** 0.5  # 8.0

NCORES = 8
ROWS_PER_CORE = 14          # query rows per core
NS = ROWS_PER_CORE * Wimg   # 784 query positions per core
NT_FULL, NT_TAIL = 6, 16    # 784 = 6*128 + 16
KT7 = 7                     # ceil(784/128) k tiles (padded to 896)
WPAD = Wimg + 2             # 58 padded cols

_CACHE = {}


def _build(nc):
    f32 = mybir.dt.float32
    bf16 = mybir.dt.bfloat16
    AF = mybir.ActivationFunctionType
    OP = mybir.AluOpType

    # ---- DRAM I/O ----
    xb16 = nc.dram_tensor("xb16", [4, 128, Himg, WPAD], bf16, kind="ExternalInput")
    xq = nc.dram_tensor("xq", [4, 128, NS], bf16, kind="ExternalInput")
    xres = nc.dram_tensor("xres", [4, 128, NS], f32, kind="ExternalInput")
    wqT = nc.dram_tensor("wqT", [4, 128, 512], bf16, kind="ExternalInput")
    wvT = nc.dram_tensor("wvT", [4, 128, 512], bf16, kind="ExternalInput")
    woT = nc.dram_tensor("woT", [4, 128, 512], bf16, kind="ExternalInput")
    bq = nc.dram_tensor("bq", [4, 128, 1], f32, kind="ExternalInput")
    bv = nc.dram_tensor("bv", [1, 512], bf16, kind="ExternalInput")
    mixT = nc.dram_tensor("mixT", [64, 2], bf16, kind="ExternalInput")
    cntR = nc.dram_tensor("cntR", [128, N2], bf16, kind="ExternalInput")
    bnsc = nc.dram_tensor("bnsc", [4, 128, 1], f32, kind="ExternalInput")
    ident = nc.dram_tensor("ident", [128, 128], bf16, kind="ExternalInput")
    out = nc.dram_tensor("out", [4, 128, NS], f32, kind="ExternalOutput")
    # scratch for partition-reshapes
    barq_d = nc.dram_tensor("barq_d", [512], bf16)
    pi_d = nc.dram_tensor("pi_d", [16], bf16)

    ctx = ExitStack()
    with ctx:
        tc = ctx.enter_context(tile.TileContext(nc))
        con = ctx.enter_context(tc.tile_pool(name="con", bufs=1))
        big = ctx.enter_context(tc.tile_pool(name="big", bufs=1))
        tmp = ctx.enter_context(tc.tile_pool(name="tmp", bufs=2))
        epool = ctx.enter_context(tc.tile_pool(name="epool", bufs=4))
        etp = ctx.enter_context(tc.tile_pool(name="etp", bufs=2))
        sps_p = ctx.enter_context(tc.tile_pool(name="sps_p", bufs=3, space="PSUM"))
        up_p = ctx.enter_context(tc.tile_pool(name="up_p", bufs=1, space="PSUM"))
        yp_p = up_p

        # ---- constants / inputs to SBUF ----
        wq_s = con.tile([128, 4, 512], bf16)
        nc.sync.dma_start(out=wq_s, in_=wqT.rearrange("k p m -> p k m"))
        ones_s = con.tile([1, 128], bf16)
        nc.vector.memset(ones_s, 1.0)
        xq_s = con.tile([128, 4, NS], bf16)
        nc.sync.dma_start(out=xq_s, in_=xq.rearrange("t p n -> p t n"))
        xb_s = con.tile([128, 4, Himg * WPAD], bf16)
        for t in range(4):
            nc.sync.dma_start(
                out=xb_s[:, t], in_=xb16.rearrange("t p h w -> p t (h w)")[:, t])
        cnt_s = con.tile([128, N2], bf16)
        nc.sync.dma_start(out=cnt_s, in_=cntR[:, :])
        wv_s = con.tile([128, 4, 512], bf16)
        nc.sync.dma_start(out=wv_s, in_=wvT.rearrange("k p m -> p k m"))
        bq_s = con.tile([128, 4], f32)
        nc.sync.dma_start(out=bq_s, in_=bq.rearrange("t p one -> p (t one)"))
        bv_s = con.tile([1, 512], bf16)
        nc.sync.dma_start(out=bv_s, in_=bv[:, :])
        mix_s = con.tile([64, 2], bf16)
        nc.sync.dma_start(out=mix_s, in_=mixT[:, :])
        bnsc_s = con.tile([128, 4], f32)
        nc.sync.dma_start(out=bnsc_s, in_=bnsc.rearrange("t p one -> p (t one)"))
        id_s = con.tile([128, 128], bf16)
        nc.sync.dma_start(out=id_s, in_=ident[:, :])

        # ---- pooling: px_norm [128, 4, N2] ----
        px_s = [big.tile([128, N2], bf16, tag=f"px{t}", name=f"px{t}")
                for t in range(4)]
        for t in range(4):
            xv = xb_s[:, t].rearrange("p (h w) -> p h w", h=Himg)  # [128,56,58]
            pw = tmp.tile([128, 58, W2], bf16, tag="pw")
            nc.vector.memset(pw[:, 0], 0.0)
            nc.vector.memset(pw[:, 57], 0.0)
            # window cols {2w2, 2w2+1, 2w2+2} in padded coords
            a0 = xv[:, :, 0:56].rearrange("p h (w two) -> p h w two", two=2)
            nc.gpsimd.tensor_add(pw[:, 1:57], a0[:, :, :, 0], a0[:, :, :, 1])
            a2 = xv[:, :, 2:58].rearrange("p h (w two) -> p h w two", two=2)
            # pw += x[2w2+2] : (x*1.0)+pw
            nc.gpsimd.tensor_add(pw[:, 1:57], a2[:, :, :, 0], pw[:, 1:57])
            ph = tmp.tile([128, H2, W2], bf16, tag="ph")
            b0 = pw.rearrange("p (h two) w -> p h two w", two=2)
            nc.vector.tensor_add(ph, b0[:, 0:28, 0], b0[:, 0:28, 1])
            nc.vector.scalar_tensor_tensor(
                out=ph, in0=pw[:, 2:58:2], scalar=1.0, in1=ph,
                op0=OP.mult, op1=OP.add)
            _px_gate = nc.vector.tensor_mul(
                px_s[t], ph.rearrange("p h w -> p (h w)"), cnt_s)

        # ---- convs ----
        _kt_gate = None
        qt_s = [big.tile([128, NS], bf16, tag=f"qt{t}", name=f"qt{t}")
                for t in range(4)]
        kt_s = [big.tile([128, N2], bf16, tag=f"kt{t}", name=f"kt{t}")
                for t in range(4)]
        for t in range(4):
            qps = sps_p.tile([128, NS], f32, tag="sc")
            for half, (c0, c1) in enumerate(((0, 512), (512, NS))):
                for kt in range(4):
                    nc.tensor.matmul(
                        qps[:, c0:c1],
                        lhsT=wq_s[:, kt, t * 128 : (t + 1) * 128],
                        rhs=xq_s[:, kt, c0:c1],
                        start=(kt == 0), stop=(kt == 3))
            nc.vector.tensor_scalar_add(qt_s[t], qps, bq_s[:, t : t + 1])
            kps = sps_p.tile([128, N2], f32, tag="sc")
            for half, (c0, c1) in enumerate(((0, 512), (512, N2))):
                for kt in range(4):
                    nc.tensor.matmul(
                        kps[:, c0:c1],
                        lhsT=wq_s[:, kt, t * 128 : (t + 1) * 128],
                        rhs=px_s[kt][:, c0:c1],
                        start=(kt == 0), stop=(kt == 3))
            _kt_gate = nc.vector.tensor_scalar_add(
                kt_s[t], kps, bq_s[:, t : t + 1])
        # vtT [n2 -> 7 tiles, 512] ; bias row replicated then added at evac
        bvR_ps = sps_p.tile([128, 512], f32, tag="sc")
        nc.tensor.matmul(bvR_ps, lhsT=ones_s, rhs=bv_s, start=True, stop=True)
        bvR_s = con.tile([128, 512], f32)
        nc.vector.tensor_copy(bvR_s, bvR_ps)
        vtT_s = [big.tile([128, 512], bf16, tag=f"vtT{mt}", name=f"vtT{mt}")
                 for mt in range(KT7)]
        for mt in range(KT7):
            mlen = 128 if mt < 6 else 16
            vps = sps_p.tile([128, 512], f32, tag="sc")
            for kt in range(4):
                nc.tensor.matmul(
                    vps[:mlen],
                    lhsT=px_s[kt][:, mt * 128 : mt * 128 + mlen],
                    rhs=wv_s[:, kt], start=(kt == 0), stop=(kt == 3))
            nc.vector.scalar_tensor_tensor(
                out=vtT_s[mt][:mlen], in0=vps[:mlen], scalar=1.0,
                in1=bvR_s[:mlen], op0=OP.mult, op1=OP.add)

        # ---- pi chain ----
        from concourse.tile import add_dep_helper as _adh
        import os as _os
        if _os.environ.get("NOPI") == "1":
            piR_s = con.tile([128, 16], f32)
            nc.vector.memset(piR_s, 0.5)
        else:
            bar32 = con.tile([128, 4], f32)
            for t in range(4):
                nc.vector.tensor_reduce(
                    out=bar32[:, t : t + 1], in_=xb_s[:, t], op=OP.add,
                    axis=mybir.AxisListType.X)
            barb = con.tile([128, 4], bf16)
            nc.vector.tensor_scalar_mul(barb, bar32, 1.0 / float(N))
            barq_ps = up_p.tile([128, 4], f32, tag="u")
            for t in range(4):
                for kt in range(4):
                    nc.tensor.matmul(
                        barq_ps[:, t : t + 1],
                        lhsT=wq_s[:, kt, t * 128 : (t + 1) * 128],
                        rhs=barb[:, kt : kt + 1],
                        start=(kt == 0), stop=(kt == 3))
            barq_s = con.tile([128, 4], bf16)
            for t in range(4):
                nc.vector.tensor_scalar_add(
                    barq_s[:, t : t + 1], barq_ps[:, t : t + 1], bq_s[:, t : t + 1])
            nc.sync.dma_start(out=barq_d.rearrange("(t p) -> p t", p=128), in_=barq_s)
            barqT = con.tile([64, 8], bf16)
            nc.sync.dma_start(out=barqT, in_=barq_d.rearrange("(h dd) -> dd h", dd=64))
            lg_ps = up_p.tile([2, 8], f32, tag="u")
            nc.tensor.matmul(lg_ps, lhsT=mix_s, rhs=barqT, start=True, stop=True)
            lg_s = con.tile([2, 8], bf16)
            nc.vector.tensor_copy(lg_s, lg_ps)
            lgT_ps = up_p.tile([8, 2], bf16, tag="u")
            nc.tensor.transpose(lgT_ps, lg_s[:, 0:8], id_s[0:2, 0:2])
            pi_e = con.tile([8, 2], f32)
            zpi = con.tile([8, 1], f32)
            nc.scalar.activation(pi_e, lgT_ps, AF.Exp, accum_out=zpi)
            rzpi = con.tile([8, 1], f32)
            nc.vector.reciprocal(rzpi, zpi)
            pi_s = con.tile([8, 2], bf16)
            nc.vector.tensor_scalar_mul(pi_s, pi_e, rzpi)
            nc.sync.dma_start(out=pi_d.rearrange("(h m) -> h m", h=8), in_=pi_s)
            piflat = con.tile([1, 16], bf16)
            nc.sync.dma_start(out=piflat, in_=pi_d.rearrange("(one f) -> one f", one=1))
            piR_ps = up_p.tile([128, 16], f32, tag="u")
            nc.tensor.matmul(piR_ps, lhsT=ones_s, rhs=piflat, start=True, stop=True)
            piR_s = con.tile([128, 16], f32)
            nc.vector.tensor_copy(piR_s, piR_ps)

        wo_s = con.tile([128, 4, 512], bf16)
        nc.sync.dma_start(out=wo_s, in_=woT.rearrange("k p m -> p k m"))
        # ---- attention per head ----
        headT_s = [big.tile([128, NS], bf16, tag=f"hT{j}", name=f"hT{j}")
                   for j in range(4)]
        ecomb_ring = [big.tile([128, 896], bf16, tag=f"ec{i}", name=f"ec{i}")
                      for i in range(3)]
        tpad_ring = [big.tile([128, NS], bf16, tag=f"tp{i}", name=f"tp{i}")
                     for i in range(3)]
        for i in range(3):
            nc.vector.memset(ecomb_ring[i][:, NS:896], 0.0)
        ecu = [0]
        for hp in range(4):
          ups = up_p.tile([128, NS], f32, tag="u", name=f"ups{hp}")
          for hm in range(2):
            h = 2 * hp + hm
            pt, pb = h // 2, (h % 2) * 64
            ET = etp.tile([128, KT7, NS], bf16, tag="ET")
            for nt in range(KT7):
                nr = 128 if nt < 6 else NT_TAIL
                n0 = nt * 128
                z2 = tmp.tile([128, 2], f32, tag="z2")
                e01 = []
                for m in range(2):
                    sps = sps_p.tile([128, NS], f32, tag="sc")
                    qb = pb + m * 32
                    for c0, c1 in ((0, 512), (512, NS)):
                        nc.tensor.matmul(
                            sps[:nr, c0:c1],
                            lhsT=qt_s[pt][qb : qb + 32, n0 : n0 + nr],
                            rhs=kt_s[pt][qb : qb + 32, c0:c1],
                            start=True, stop=True, tile_position=(qb, 0))
                    em = epool.tile([128, NS], bf16, tag="em")
                    nc.scalar.activation(
                        em[:nr], sps[:nr], AF.Exp, scale=1.0 / TEMP,
                        accum_out=z2[:nr, m : m + 1])
                    e01.append(em)
                rz = tmp.tile([128, 2], f32, tag="rz")
                nc.vector.reciprocal(rz[:nr], z2[:nr])
                bt = tmp.tile([128, 2], f32, tag="bt")
                nc.vector.tensor_mul(bt[:nr], rz[:nr], piR_s[:nr, 2 * h : 2 * h + 2])
                ecomb = ecomb_ring[ecu[0] % 3]
                t1 = tpad_ring[ecu[0] % 3]
                ecu[0] += 1
                nc.gpsimd.tensor_scalar_mul(
                    ecomb[:nr, 0:NS], e01[0][:nr], bt[:nr, 0:1])
                nc.gpsimd.tensor_scalar_mul(t1[:nr], e01[1][:nr], bt[:nr, 1:2])
                nc.vector.tensor_add(
                    ecomb[:nr, 0:NS], ecomb[:nr, 0:NS], t1[:nr])
                nc.sync.dma_start(
                    out=ET[:, :, n0 : n0 + nr],
                    in_=ecomb[:nr, :].rearrange("a (k b) -> a k b", k=KT7),
                    transpose=True)
            for c0, c1 in ((0, 512), (512, NS)):
                for kt in range(KT7):
                    klen = 128 if kt < 6 else NT_TAIL
                    nc.tensor.matmul(
                        ups[pb : pb + 64, c0:c1],
                        lhsT=vtT_s[kt][:klen, h * 64 : h * 64 + 64],
                        rhs=ET[:klen, kt, c0:c1],
                        start=(kt == 0), stop=(kt == 6), skip_group_check=True)
            if h % 2 == 1:
                nc.vector.tensor_copy(headT_s[h // 2], ups)  # noqa

        xres_s = con.tile([128, 4, NS], f32)
        nc.sync.dma_start(out=xres_s, in_=xres.rearrange("t p n -> p t n"))
        # ---- output projection + bn + residual ----
        out_s = big.tile([128, 4, NS], f32)
        for ct in range(4):
            yps = sps_p.tile([128, NS], f32, tag="sc")
            for c0, c1 in ((0, 512), (512, NS)):
                for kt in range(4):
                    nc.tensor.matmul(
                        yps[:, c0:c1],
                        lhsT=wo_s[:, kt, ct * 128 : (ct + 1) * 128],
                        rhs=headT_s[kt][:, c0:c1],
                        start=(kt == 0), stop=(kt == 3))
            nc.vector.scalar_tensor_tensor(
                out=out_s[:, ct], in0=yps, scalar=bnsc_s[:, ct : ct + 1],
                in1=xres_s[:, ct], op0=OP.mult, op1=OP.add)
            nc.sync.dma_start(
                out=out.rearrange("t p n -> p t n")[:, ct], in_=out_s[:, ct])
    nc.finalize()
    return nc


def _prep_host(inputs):
    x = np.asarray(inputs["x"], np.float32)
    w_qs = np.asarray(inputs["w_qs"], np.float32)
    b_qs = np.asarray(inputs["b_qs"], np.float32)
    w_vs = np.asarray(inputs["w_vs"], np.float32)
    b_vs = np.asarray(inputs["b_vs"], np.float32)
    mix_w = np.asarray(inputs["mix_w"], np.float32)
    w_out = np.asarray(inputs["w_out"], np.float32)
    gam = np.asarray(inputs["bn_gamma"], np.float32)
    bet = np.asarray(inputs["bn_beta"], np.float32)

    inv = 1.0 / np.sqrt(np.float32(1.0 + BN_EPS))
    bnsc = (gam * inv).reshape(4, 128, 1)
    # pooling count reciprocal, replicated across partitions
    cw = np.full(W2, 3.0, np.float32); cw[0] = 2.0
    chh = np.full(H2, 3.0, np.float32); chh[0] = 2.0
    cnt = (chh[:, None] * cw[None, :]).reshape(-1)
    cntR = np.broadcast_to((1.0 / cnt)[None, :], (128, N2)).astype(BF16)

    shared = dict(
        wqT=w_qs.T.reshape(4, 128, 512).astype(BF16),
        wvT=w_vs.T.reshape(4, 128, 512).astype(BF16),
        woT=w_out.T.reshape(4, 128, 512).astype(BF16),
        bq=b_qs.reshape(4, 128, 1),
        bv=b_vs.reshape(1, 512).astype(BF16),
        mixT=mix_w.T.astype(BF16),
        cntR=np.ascontiguousarray(cntR),
        bnsc=bnsc,
        ident=np.eye(128, dtype=BF16),
    )
    xf = x.reshape(B, C, Himg, Wimg)
    xpad = np.zeros((B, 4, 128, Himg, WPAD), np.float32)
    xpad[:, :, :, :, 1:57] = xf.reshape(B, 4, 128, Himg, Wimg)
    xpad16 = xpad.astype(BF16)
    in_maps = []
    for c in range(NCORES):
        b, r = c // 4, c % 4
        sl = xf[b, :, 14 * r : 14 * r + 14, :].reshape(C, NS)
        m = dict(shared)
        m["xb16"] = np.ascontiguousarray(xpad16[b])
        m["xq"] = np.ascontiguousarray(sl.reshape(4, 128, NS).astype(BF16))
        m["xres"] = np.ascontiguousarray(
            (sl + bet[:, None]).reshape(4, 128, NS).astype(np.float32))
        in_maps.append(m)
    return in_maps


def kernel(**inputs):
    from concourse.bass_utils import run_bass_kernel_spmd

    if "nc" not in _CACHE:
        _CACHE["nc"] = _build(bacc.Bacc())
    nc = _CACHE["nc"]
    in_maps = _prep_host(inputs)
    import os
    trace = os.environ.get("KTRACE") == "1"
    t0 = time.time()
    res = run_bass_kernel_spmd(nc, in_maps, core_ids=list(range(NCORES)),
                               trace=trace)
    _CACHE["wall"] = time.time() - t0
    if getattr(res, "exec_time_ns", None):
        _CACHE["hw_ns"] = res.exec_time_ns
    _CACHE["res_obj"] = res
    outs = res.results if hasattr(res, "results") else res
    y = np.zeros((B, C, Himg, Wimg), np.float32)
    for c in range(NCORES):
        b, r = c // 4, c % 4
        oc = np.asarray(outs[c]["out"], np.float32).reshape(C, 14, Wimg)
        y[b, :, 14 * r : 14 * r + 14, :] = oc
    return y

